# revision 1
# baseline (speedup 1.0000x reference)
"""ECE loss kernel for Trainium2, data-parallel over 8 NeuronCores.

Host side shards + permutes samples (the binning is permutation invariant)
into 128-sample single-label "slots" so the device never needs a per-sample
label gather: the accuracy test becomes a strided column read baked into the
access pattern.  Device computes per-sample confidence (no-max-subtraction
softmax is safe for N(0,1) logits), bins via 15 threshold compares, and
accumulates per-bin (sum_conf, sum_acc) with one PE matmul per tile.  The
final ECE is sum_b |sum_conf_b - sum_acc_b| / N, all-reduced across cores.
"""

import dataclasses
import hashlib
import sys

import numpy as np

sys.path.insert(0, "/opt/trn_rl_repo")

from concourse import bacc, bass, mybir, tile  # noqa: E402
from concourse import bass_utils  # noqa: E402

P = 128          # partitions
SPP = 32         # samples per partition per tile (groups/slots per tile)
TILE = P * SPP   # samples per tile
C = 100          # classes
NBINS = 15
N_CORES = 8
BIG = 80.0       # pad-row logit; exp(80) finite in f32, exp(-80) -> 0
N_TOTAL = 2_000_000
K_ACT = 6        # groups/tile whose exp+sum runs fused on ScalarE (rest: DVE)
DMA_PAIR = 2     # logical tiles loaded per dma_start (bigger rows, better BW)

F32 = mybir.dt.float32
AX = mybir.AxisListType
ALU = mybir.AluOpType
ACTF = mybir.ActivationFunctionType


# ---------------------------------------------------------------- host layout

def build_plan(labels: np.ndarray, n_cores: int = N_CORES):
    """Deal samples round-robin per label so every core has the same number
    of 128-sample slots per label.  Returns (slot_labels, per-core sample
    index arrays with -1 for pad rows)."""
    labels = np.asarray(labels).astype(np.int64).ravel()
    order = np.argsort(labels, kind="stable")
    sorted_labels = labels[order]
    # block boundaries per label
    starts = np.searchsorted(sorted_labels, np.arange(C))
    ends = np.searchsorted(sorted_labels, np.arange(C), side="right")

    slot_labels = []
    core_chunks = [[] for _ in range(n_cores)]
    for k in range(C):
        idx_k = order[starts[k]:ends[k]]
        # core c takes idx_k[c::n_cores]
        per_core = [idx_k[c::n_cores] for c in range(n_cores)]
        max_cnt = max(len(x) for x in per_core)
        slots_k = max(1, -(-max_cnt // P)) if max_cnt > 0 else 0
        if slots_k == 0:
            continue
        padded = slots_k * P
        for c in range(n_cores):
            buf = np.full(padded, -1, dtype=np.int64)
            buf[: len(per_core[c])] = per_core[c]
            core_chunks[c].append(buf)
        slot_labels.extend([k] * slots_k)

    n_slots = len(slot_labels)
    # pad slot count to a full DMA-pair multiple (pad slots use label 0)
    pad_slots = (-n_slots) % (SPP * DMA_PAIR)
    if pad_slots:
        for c in range(n_cores):
            core_chunks[c].append(np.full(pad_slots * P, -1, dtype=np.int64))
        slot_labels.extend([0] * pad_slots)
        n_slots += pad_slots

    slot_labels = np.asarray(slot_labels, dtype=np.int64)
    core_idx = [np.concatenate(ch) for ch in core_chunks]
    T = n_slots // SPP
    return slot_labels, core_idx, T


def label_runs(slot_labels: np.ndarray, T: int):
    """Per tile: list of (g0, g1, k) runs of equal-label slots."""
    runs = []
    for t in range(T):
        ks = slot_labels[t * SPP:(t + 1) * SPP]
        tile_runs = []
        g0 = 0
        for g in range(1, SPP + 1):
            if g == SPP or ks[g] != ks[g0]:
                tile_runs.append((g0, g, int(ks[g0])))
                g0 = g
        runs.append(tile_runs)
    return runs


def build_core_slab(logits: np.ndarray, idx: np.ndarray,
                    slot_labels: np.ndarray) -> np.ndarray:
    """Materialize one core's [T*TILE, C] f32 slab in device tile order:
    row (t*TILE + p*SPP + g) holds the p-th sample of slot t*SPP+g."""
    S = len(slot_labels)
    arr = logits[np.maximum(idx, 0)].astype(np.float32, copy=True)
    pad_pos = np.nonzero(idx < 0)[0]
    if len(pad_pos):
        ks = slot_labels[pad_pos // P]
        arr[pad_pos] = -BIG
        arr[pad_pos, ks] = BIG
    # [S, P, C] slot-major -> [Tpair, DMA_PAIR, SPP, P, C] -> pair-DMA order
    # [Tpair, P, DMA_PAIR, SPP, C]: each partition holds DMA_PAIR*SPP
    # consecutive samples of one pair-load.
    arr = arr.reshape(S // (SPP * DMA_PAIR), DMA_PAIR, SPP, P, C)
    arr = arr.transpose(0, 3, 1, 2, 4)
    return np.ascontiguousarray(arr).reshape(-1, C)


# ------------------------------------------------------------- device program

def _bcast(ap, extra):
    """Append a step-0 (broadcast) dim of size `extra` to an AP."""
    return dataclasses.replace(ap, ap=ap.ap + [[0, extra]])


def build_program(T: int, runs, n_total: int, n_cores: int = N_CORES):
    nc = bacc.Bacc("TRN2", target_bir_lowering=False, debug=False,
                   num_devices=n_cores)

    logits_d = nc.dram_tensor("logits", [T * TILE, C], F32, kind="ExternalInput")
    tempr_d = nc.dram_tensor("tempr", [P, 1], F32, kind="ExternalInput")
    thr_d = nc.dram_tensor("thr", [P, SPP * NBINS], F32, kind="ExternalInput")
    wvec_d = nc.dram_tensor("wvec", [2, 1], F32, kind="ExternalInput")
    out_d = nc.dram_tensor("out", [1], F32, kind="ExternalOutput")

    with tile.TileContext(nc) as tc:
        with (
            tc.tile_pool(name="const", bufs=1) as const,
            tc.tile_pool(name="rawp", bufs=3) as rawp,
            tc.tile_pool(name="sb", bufs=3) as sbp,
            tc.tile_pool(name="psH", bufs=1, space="PSUM") as psH,
            tc.tile_pool(name="psF", bufs=1, space="PSUM") as psF,
            tc.tile_pool(name="dram", bufs=1, space="DRAM") as dram,
        ):
            tempr_t = const.tile([P, 1], F32)
            nc.sync.dma_start(tempr_t, tempr_d.ap())
            thr_t = const.tile([P, SPP * NBINS], F32)
            nc.sync.dma_start(thr_t, thr_d.ap())
            wvec_t = const.tile([2, 1], F32)
            nc.sync.dma_start(wvec_t, wvec_d.ap())
            invT = const.tile([P, 1], F32)
            nc.vector.reciprocal(invT, tempr_t)

            hist = psH.tile([2 * SPP, SPP * NBINS], F32)

            assert T % DMA_PAIR == 0
            logits_ap = logits_d.ap()
            for t in range(T):
                h = t % DMA_PAIR
                if h == 0:
                    rawp_t = rawp.tile([P, DMA_PAIR * SPP * C], F32,
                                       tag="raw", name="rawp_t")
                    src = logits_ap[t * TILE:(t + DMA_PAIR) * TILE,
                                    :].rearrange("(p s) c -> p (s c)", p=P)
                    nc.sync.dma_start(rawp_t, src)
                raw = rawp_t[:, h * SPP * C:(h + 1) * SPP * C]

                raw3 = raw.rearrange("p (g c) -> p g c", g=SPP)
                m = sbp.tile([P, SPP], F32, tag="m", name="m", bufs=4)
                nc.vector.reduce_max(m, raw3, axis=AX.X)

                # denominators: ScalarE handles K_ACT groups with fused
                # exp+accum; DVE 3D-sums the rest over one big-FD exp.
                D = sbp.tile([P, SPP], F32, tag="D", name="D", bufs=4)
                for g in range(K_ACT):
                    expg = sbp.tile([P, C], F32, tag="expg", name="expg", bufs=4)
                    nc.scalar.activation(expg, raw[:, g * C:(g + 1) * C],
                                         ACTF.Exp, scale=invT,
                                         accum_out=D[:, g:g + 1])
                if K_ACT < SPP:
                    nd = SPP - K_ACT
                    expA = sbp.tile([P, nd * C], F32, tag="expA", name="expA")
                    nc.scalar.activation(expA, raw[:, K_ACT * C:], ACTF.Exp,
                                         scale=invT)
                    nc.vector.reduce_sum(
                        D[:, K_ACT:SPP],
                        expA.rearrange("p (g c) -> p g c", g=nd), axis=AX.X)

                rd = sbp.tile([P, SPP], F32, tag="rd", name="rd", bufs=4)
                nc.vector.reciprocal(rd, D)
                expm = sbp.tile([P, SPP], F32, tag="expm", name="expm", bufs=4)
                nc.scalar.activation(expm, m, ACTF.Exp, scale=invT)

                pack = sbp.tile([P, 2 * SPP], F32, tag="pack", name="pack", bufs=4)
                nc.vector.tensor_tensor(pack[:, 0:2 * SPP:2], expm, rd,
                                        op=ALU.mult)
                for (g0, g1, k) in runs[t]:
                    lab = raw3[:, g0:g1, k:k + 1].opt()
                    nc.vector.tensor_tensor(
                        pack[:, 2 * g0 + 1:2 * g1:2], lab,
                        m[:, g0:g1], op=ALU.is_ge)

                mask = sbp.tile([P, SPP * NBINS], F32, tag="mask", name="mask", bufs=4)
                conf_b = _bcast(pack[:, 0:2 * SPP:2], NBINS)
                thr3 = thr_t.rearrange("p (g b) -> p g b", g=SPP)
                mask3 = mask.rearrange("p (g b) -> p g b", g=SPP)
                nc.vector.tensor_tensor(mask3, conf_b, thr3, op=ALU.is_gt)

                nc.tensor.matmul(hist, lhsT=pack, rhs=mask,
                                 start=(t == 0), stop=(t == T - 1))

            # ---- finalize: collapse diagonal blocks, cum->bin, allreduce
            hist_sb = sbp.tile([2 * SPP, SPP * NBINS], F32)
            nc.vector.tensor_copy(hist_sb, hist)
            stats3 = sbp.tile([2, SPP * NBINS], F32)
            for q in range(SPP):
                nc.sync.dma_start(
                    stats3[:, q * NBINS:(q + 1) * NBINS],
                    hist_sb[2 * q:2 * q + 2, q * NBINS:(q + 1) * NBINS])
            cum = sbp.tile([2, NBINS], F32)
            nc.vector.reduce_sum(
                cum, stats3.rearrange("p (q b) -> p b q", q=SPP), axis=AX.X)
            cum16 = sbp.tile([2, NBINS + 1], F32)
            nc.vector.memset(cum16, 0.0)
            nc.vector.tensor_copy(cum16[:, 0:NBINS], cum)
            bstats = sbp.tile([2, NBINS], F32)
            nc.vector.tensor_tensor(bstats, cum16[:, 0:NBINS],
                                    cum16[:, 1:NBINS + 1], op=ALU.subtract)

            cc_in = dram.tile([2, NBINS], F32)
            cc_out = dram.tile([2, NBINS], F32)
            nc.sync.dma_start(cc_in, bstats)
            nc.gpsimd.collective_compute(
                "AllReduce", ALU.add,
                replica_groups=[list(range(n_cores))],
                ins=[cc_in.opt()], outs=[cc_out.opt()])
            ar = sbp.tile([2, NBINS], F32)
            nc.sync.dma_start(ar, cc_out)

            dd = psF.tile([1, NBINS], F32)
            nc.tensor.matmul(dd, lhsT=wvec_t, rhs=ar, start=True, stop=True)
            esum = sbp.tile([1, 1], F32)
            nc.vector.tensor_reduce(esum, dd, axis=AX.X, op=ALU.add,
                                    apply_absolute_value=True)
            res = sbp.tile([1, 1], F32)
            nc.scalar.mul(res, esum, 1.0 / n_total)
            nc.sync.dma_start(out_d.ap(), res)

    nc.compile()
    return nc


# ------------------------------------------------------------------- runner

def make_const_inputs():
    thr = np.tile((np.arange(NBINS, dtype=np.float32) / np.float32(NBINS)),
                  SPP)
    return {
        "thr": np.broadcast_to(thr, (P, SPP * NBINS)).copy(),
        "wvec": np.array([[1.0], [-1.0]], np.float32),
    }


_CACHE = {}


def _prepare(logits, labels, temperature, n_total, n_cores=N_CORES):
    labels = np.asarray(labels)
    key = hashlib.sha1(labels.tobytes()).hexdigest()
    if key in _CACHE:
        nc, slot_labels, core_idx, T = _CACHE[key]
    else:
        slot_labels, core_idx, T = build_plan(labels, n_cores)
        nc = build_program(T, label_runs(slot_labels, T), n_total, n_cores)
        _CACHE[key] = (nc, slot_labels, core_idx, T)

    logits = np.asarray(logits, dtype=np.float32)
    consts = make_const_inputs()
    tempr = np.broadcast_to(
        np.asarray(temperature, np.float32).ravel()[0:1], (P, 1)).copy()
    in_maps = []
    for c in range(n_cores):
        m = dict(consts)
        m["tempr"] = tempr
        m["logits"] = build_core_slab(logits, core_idx[c], slot_labels)
        in_maps.append(m)
    return nc, in_maps


def _ensure_ntff_hook():
    """This container's antenv lacks axon_hooks; synthesize it and register
    the ctypes NTFF hook so trace=True works under axon."""
    try:
        import antenv.axon_hooks  # noqa: F401
        return
    except ImportError:
        pass
    import types

    import antenv

    mod = types.ModuleType("antenv.axon_hooks")
    _hook = [None]
    mod.set_axon_ntff_profile_hook = lambda h: _hook.__setitem__(0, h)
    mod.get_axon_ntff_profile_hook = lambda: _hook[0]
    sys.modules["antenv.axon_hooks"] = mod
    antenv.axon_hooks = mod
    try:
        from trn_agent_boot.trn_boot import _ntff_profile_via_ctypes
        mod.set_axon_ntff_profile_hook(
            _ntff_profile_via_ctypes("/opt/axon/libaxon_pjrt.so"))
    except Exception:
        pass


def run(logits, labels, temperature, n_total=None, trace=False,
        n_cores=N_CORES):
    if trace:
        _ensure_ntff_hook()
    if n_total is None:
        n_total = int(np.asarray(labels).shape[0])
    nc, in_maps = _prepare(logits, labels, temperature, n_total, n_cores)
    res = bass_utils.run_bass_kernel_spmd(
        nc, in_maps, core_ids=list(range(n_cores)), trace=trace)
    out = np.asarray(res.results[0]["out"], dtype=np.float32).reshape(1)
    return out, res


def kernel(logits, labels, temperature):
    out, _ = run(logits, labels, temperature)
    return out



# revision 2
# speedup vs baseline: 1.3913x; 1.3913x over previous
"""ECE loss kernel for Trainium2, data-parallel over 8 NeuronCores.

Host side shards + permutes samples (binning is permutation invariant) into
128-sample single-label "slots" so the device never needs a per-sample label
gather, and casts logits to bf16 (ECE rel-err from bf16 ~3e-3, tolerance 2e-2).

Device per mega-tile (128 partitions x 64 slots x 104 padded classes, bf16):
  ScalarE: exp(logits/T); K_ACT groups use fused accum for the softmax
           denominator, the rest get one big exp instruction.
  DVE:     pairwise tensor_tensor max/add trees (bf16 2x mode) for per-sample
           max-exp and the remaining denominators, fast reciprocal, conf/acc,
           cumulative threshold mask.
  PE:      one 128-wide matmul pair accumulates per-(slot, bin) sums of
           (conf, acc) into PSUM across all mega-tiles.
Each core DMAs its [2,128,480] histogram out; the host extracts the diagonal
slot blocks, reduces 8 cores' 15-bin stats, and finishes ECE in float64.
"""

import dataclasses
import hashlib
import sys

import numpy as np

sys.path.insert(0, "/opt/trn_rl_repo")

import ml_dtypes  # noqa: E402

from concourse import bacc, bass, mybir, tile  # noqa: E402
from concourse import bass_utils  # noqa: E402

P = 128          # partitions
G = 64           # slots (groups) per mega-tile
MEGA = P * G     # samples per mega-tile
C = 100          # classes
CP = 104         # padded class stride (4B-aligned bf16 rows, tree-friendly)
NBINS = 15
N_CORES = 8
BIG = 80.0       # pad logit; exp(-80) ~ 1.8e-35 is harmless in f32/bf16
N_TOTAL = 2_000_000
K_ACT = 14       # groups/mega whose exp+denominator runs fused on ScalarE
DMAP = 2         # mega-tiles per dma_start (3.4 MB transfers)

F32 = mybir.dt.float32
BF16 = mybir.dt.bfloat16
BFNP = ml_dtypes.bfloat16
AX = mybir.AxisListType
ALU = mybir.AluOpType
ACTF = mybir.ActivationFunctionType


# ---------------------------------------------------------------- host layout

def build_plan(labels: np.ndarray, n_cores: int = N_CORES):
    """Deal samples round-robin per label so every core has the same number
    of 128-sample slots per label.  Returns (slot_labels, per-core sample
    index arrays with -1 for pad rows, mega-tile count)."""
    labels = np.asarray(labels).astype(np.int64).ravel()
    order = np.argsort(labels, kind="stable")
    sorted_labels = labels[order]
    starts = np.searchsorted(sorted_labels, np.arange(C))
    ends = np.searchsorted(sorted_labels, np.arange(C), side="right")

    slot_labels = []
    core_chunks = [[] for _ in range(n_cores)]
    for k in range(C):
        idx_k = order[starts[k]:ends[k]]
        per_core = [idx_k[c::n_cores] for c in range(n_cores)]
        max_cnt = max(len(x) for x in per_core)
        slots_k = -(-max_cnt // P) if max_cnt > 0 else 0
        if slots_k == 0:
            continue
        padded = slots_k * P
        for c in range(n_cores):
            buf = np.full(padded, -1, dtype=np.int64)
            buf[: len(per_core[c])] = per_core[c]
            core_chunks[c].append(buf)
        slot_labels.extend([k] * slots_k)

    n_slots = len(slot_labels)
    pad_slots = (-n_slots) % (G * DMAP)
    if pad_slots:
        for c in range(n_cores):
            core_chunks[c].append(np.full(pad_slots * P, -1, dtype=np.int64))
        slot_labels.extend([0] * pad_slots)
        n_slots += pad_slots

    slot_labels = np.asarray(slot_labels, dtype=np.int64)
    core_idx = [np.concatenate(ch) for ch in core_chunks]
    M = n_slots // G
    return slot_labels, core_idx, M


def label_runs(slot_labels: np.ndarray, M: int):
    """Per mega-tile: list of (g0, g1, k) runs of equal-label slots."""
    runs = []
    for t in range(M):
        ks = slot_labels[t * G:(t + 1) * G]
        tile_runs = []
        g0 = 0
        for g in range(1, G + 1):
            if g == G or ks[g] != ks[g0]:
                tile_runs.append((g0, g, int(ks[g0])))
                g0 = g
        runs.append(tile_runs)
    return runs


def build_core_slab(logits: np.ndarray, idx: np.ndarray,
                    slot_labels: np.ndarray) -> np.ndarray:
    """One core's [M*MEGA, CP] bf16 slab in device DMA order: within a
    DMAP-mega load, partition p holds DMAP*G consecutive samples' rows."""
    S = len(slot_labels)
    arr = np.full((S * P, CP), -BIG, dtype=BFNP)
    arr[:, :C] = logits[np.maximum(idx, 0)].astype(BFNP)
    pad_pos = np.nonzero(idx < 0)[0]
    if len(pad_pos):
        ks = slot_labels[pad_pos // P]
        arr[pad_pos, :C] = BFNP(-BIG)
        arr[pad_pos, ks] = BFNP(BIG)
    # [S, P, CP] slot-major -> [Mpair, P, DMAP, G, CP]
    arr = arr.reshape(S // (G * DMAP), DMAP, G, P, CP)
    arr = arr.transpose(0, 3, 1, 2, 4)
    return np.ascontiguousarray(arr).reshape(-1, CP)


# ------------------------------------------------------------- device program

def _bcast(ap, extra):
    """Append a step-0 (broadcast) dim of size `extra` to an AP."""
    return dataclasses.replace(ap, ap=ap.ap + [[0, extra]])


def build_program(M: int, runs, n_cores: int = N_CORES):
    nc = bacc.Bacc("TRN2", target_bir_lowering=False, debug=False,
                   num_devices=n_cores)

    logits_d = nc.dram_tensor("logits", [M * MEGA, CP], BF16,
                              kind="ExternalInput")
    tempr_d = nc.dram_tensor("tempr", [P, 1], F32, kind="ExternalInput")
    thr_d = nc.dram_tensor("thr", [P, G * NBINS], BF16, kind="ExternalInput")
    out_d = nc.dram_tensor("out", [2, P, 32 * NBINS], F32,
                           kind="ExternalOutput")

    ND = G - K_ACT  # groups whose denominator comes from the DVE sum tree

    with tile.TileContext(nc) as tc:
        with (
            tc.tile_pool(name="const", bufs=1) as const,
            tc.tile_pool(name="rawp", bufs=2) as rawp,
            tc.tile_pool(name="sb", bufs=3) as sbp,
            tc.tile_pool(name="ps", bufs=1, space="PSUM") as psp,
        ):
            tempr_t = const.tile([P, 1], F32)
            nc.sync.dma_start(tempr_t, tempr_d.ap())
            thr_t = const.tile([P, G * NBINS], BF16)
            nc.sync.dma_start(thr_t, thr_d.ap())
            invT = const.tile([P, 1], F32)
            nc.vector.reciprocal(invT, tempr_t)

            hist1 = psp.tile([P, 32 * NBINS], F32)
            hist2 = psp.tile([P, 32 * NBINS], F32)

            assert M % DMAP == 0
            logits_ap = logits_d.ap()
            for t in range(M):
                d = t % DMAP
                if d == 0:
                    rawp_t = rawp.tile([P, DMAP * G * CP], BF16,
                                       tag="raw", name="rawp_t")
                    src = logits_ap[t * MEGA:(t + DMAP) * MEGA,
                                    :].rearrange("(p s) c -> p (s c)", p=P)
                    nc.sync.dma_start(rawp_t, src)
                raw = rawp_t[:, d * G * CP:(d + 1) * G * CP]

                # ---- ScalarE: exp(l/T); K_ACT fused denominators
                E = sbp.tile([P, G * CP], BF16, tag="E", name="E", bufs=3)
                S = sbp.tile([P, G], F32, tag="S", name="S")
                for g in range(K_ACT):
                    nc.scalar.activation(E[:, g * CP:(g + 1) * CP],
                                         raw[:, g * CP:(g + 1) * CP],
                                         ACTF.Exp, scale=invT,
                                         accum_out=S[:, g:g + 1])
                nc.scalar.activation(E[:, K_ACT * CP:], raw[:, K_ACT * CP:],
                                     ACTF.Exp, scale=invT)

                E3 = E.rearrange("p (g c) -> p g c", g=G)

                # ---- DVE: max tree over all groups (bf16 2x TT ops)
                t1 = sbp.tile([P, G * 52], BF16, tag="t1", name="t1")
                t13 = t1.rearrange("p (g c) -> p g c", g=G)
                nc.vector.tensor_tensor(t13, E3[:, :, 0:52], E3[:, :, 52:104],
                                        op=ALU.max)
                t2 = sbp.tile([P, G * 26], BF16, tag="t2", name="t2")
                t23 = t2.rearrange("p (g c) -> p g c", g=G)
                nc.vector.tensor_tensor(t23, t13[:, :, 0:26], t13[:, :, 26:52],
                                        op=ALU.max)
                t3 = sbp.tile([P, G * 14], BF16, tag="t3", name="t3")
                t33 = t3.rearrange("p (g c) -> p g c", g=G)
                nc.vector.tensor_tensor(t33, t23[:, :, 0:14], t23[:, :, 12:26],
                                        op=ALU.max)
                emax = sbp.tile([P, G], BF16, tag="emax", name="emax")
                nc.vector.reduce_max(emax, t33, axis=AX.X)

                # ---- DVE: sum tree for the remaining denominators
                EK = E3[:, K_ACT:, :]
                u1 = sbp.tile([P, ND * 52], BF16, tag="u1", name="u1")
                u13 = u1.rearrange("p (g c) -> p g c", g=ND)
                nc.vector.tensor_tensor(u13, EK[:, :, 0:52], EK[:, :, 52:104],
                                        op=ALU.add)
                u2 = sbp.tile([P, ND * 26], BF16, tag="u2", name="u2")
                u23 = u2.rearrange("p (g c) -> p g c", g=ND)
                nc.vector.tensor_tensor(u23, u13[:, :, 0:26], u13[:, :, 26:52],
                                        op=ALU.add)
                u3 = sbp.tile([P, ND * 14], BF16, tag="u3", name="u3")
                u33 = u3.rearrange("p (g c) -> p g c", g=ND)
                nc.vector.tensor_tensor(u33[:, :, 0:12], u23[:, :, 0:12],
                                        u23[:, :, 14:26], op=ALU.add)
                nc.vector.tensor_copy(u33[:, :, 12:14], u23[:, :, 12:14])
                nc.vector.reduce_sum(S[:, K_ACT:], u33, axis=AX.X)

                # ---- per-sample: conf = emax / S; acc = E[label] >= emax
                R = sbp.tile([P, G], F32, tag="R", name="R")
                nc.vector.reciprocal_approx_fast(R, S)
                Rb = sbp.tile([P, G], BF16, tag="Rb", name="Rb")
                nc.vector.tensor_copy(Rb, R)
                pack = sbp.tile([P, 2 * G], BF16, tag="pack", name="pack")
                nc.vector.tensor_tensor(pack[:, 0:G], emax, Rb, op=ALU.mult)
                for (g0, g1, k) in runs[t]:
                    lab = E3[:, g0:g1, k:k + 1].opt()
                    nc.vector.tensor_tensor(pack[:, G + g0:G + g1], lab,
                                            emax[:, g0:g1], op=ALU.is_ge)

                # ---- cumulative bin mask + histogram matmuls
                mask = sbp.tile([P, G * NBINS], BF16, tag="mask", name="mask")
                conf_b = _bcast(pack[:, 0:G], NBINS)
                thr3 = thr_t.rearrange("p (g b) -> p g b", g=G)
                mask3 = mask.rearrange("p (g b) -> p g b", g=G)
                nc.vector.tensor_tensor(mask3, conf_b, thr3, op=ALU.is_gt)

                nc.tensor.matmul(hist1, lhsT=pack, rhs=mask[:, 0:32 * NBINS],
                                 start=(t == 0), stop=(t == M - 1))
                nc.tensor.matmul(hist2, lhsT=pack, rhs=mask[:, 32 * NBINS:],
                                 start=(t == 0), stop=(t == M - 1))

            # ---- finalize: dump both histograms; host does the reduction
            hist_sb = sbp.tile([P, 32 * NBINS], F32, tag="hsb", name="hsb",
                               bufs=2)
            nc.vector.tensor_copy(hist_sb, hist1)
            nc.sync.dma_start(out_d.ap()[0], hist_sb)
            hist_sb2 = sbp.tile([P, 32 * NBINS], F32, tag="hsb", name="hsb2",
                                bufs=2)
            nc.vector.tensor_copy(hist_sb2, hist2)
            nc.sync.dma_start(out_d.ap()[1], hist_sb2)

    nc.compile()
    return nc


# ------------------------------------------------------------------- runner

def make_const_inputs():
    thr = np.tile((np.arange(NBINS, dtype=np.float32) / np.float32(NBINS)),
                  G).astype(BFNP)
    return {"thr": np.broadcast_to(thr, (P, G * NBINS)).copy()}


_CACHE = {}


def _prepare(logits, labels, temperature, n_cores=N_CORES):
    labels = np.asarray(labels)
    key = hashlib.sha1(labels.tobytes()).hexdigest()
    if key in _CACHE:
        nc, slot_labels, core_idx, M = _CACHE[key]
    else:
        slot_labels, core_idx, M = build_plan(labels, n_cores)
        nc = build_program(M, label_runs(slot_labels, M), n_cores)
        _CACHE[key] = (nc, slot_labels, core_idx, M)

    logits = np.asarray(logits, dtype=np.float32)
    consts = make_const_inputs()
    tempr = np.broadcast_to(
        np.asarray(temperature, np.float32).ravel()[0:1], (P, 1)).copy()
    in_maps = []
    for c in range(n_cores):
        m = dict(consts)
        m["tempr"] = tempr
        m["logits"] = build_core_slab(logits, core_idx[c], slot_labels)
        in_maps.append(m)
    return nc, in_maps


def finalize_host(hists, n_total=N_TOTAL):
    """hists: list of per-core [2, P, 32*NBINS] f32. Returns ECE f32 [1]."""
    qs = np.arange(32)
    sc_cum = np.zeros(NBINS, np.float64)
    sa_cum = np.zeros(NBINS, np.float64)
    for h in hists:
        h = np.asarray(h, np.float64).reshape(2, P, 32, NBINS)
        for j in range(2):
            # conf rows are pack cols 32j..32j+32; acc rows 64+32j..
            sc_cum += h[j, 32 * j + qs, qs, :].sum(axis=0)
            sa_cum += h[j, 64 + 32 * j + qs, qs, :].sum(axis=0)
    sc = sc_cum - np.concatenate([sc_cum[1:], [0.0]])
    sa = sa_cum - np.concatenate([sa_cum[1:], [0.0]])
    ece = np.abs(sc - sa).sum() / float(n_total)
    return np.asarray([ece], dtype=np.float32)


def _ensure_ntff_hook():
    """This container's antenv lacks axon_hooks; synthesize it and register
    the ctypes NTFF hook so trace=True works under axon."""
    try:
        import antenv.axon_hooks  # noqa: F401
        return
    except ImportError:
        pass
    import types

    import antenv

    mod = types.ModuleType("antenv.axon_hooks")
    _hook = [None]
    mod.set_axon_ntff_profile_hook = lambda h: _hook.__setitem__(0, h)
    mod.get_axon_ntff_profile_hook = lambda: _hook[0]
    sys.modules["antenv.axon_hooks"] = mod
    antenv.axon_hooks = mod
    try:
        from trn_agent_boot.trn_boot import _ntff_profile_via_ctypes
        mod.set_axon_ntff_profile_hook(
            _ntff_profile_via_ctypes("/opt/axon/libaxon_pjrt.so"))
    except Exception:
        pass


def run(logits, labels, temperature, n_total=None, trace=False,
        n_cores=N_CORES):
    if trace:
        _ensure_ntff_hook()
    if n_total is None:
        n_total = int(np.asarray(labels).shape[0])
    nc, in_maps = _prepare(logits, labels, temperature, n_cores)
    res = bass_utils.run_bass_kernel_spmd(
        nc, in_maps, core_ids=list(range(n_cores)), trace=trace)
    out = finalize_host([r["out"] for r in res.results], n_total)
    return out, res


def kernel(logits, labels, temperature):
    out, _ = run(logits, labels, temperature)
    return out


# revision 5
# speedup vs baseline: 1.7262x; 1.2407x over previous
"""ECE loss kernel for Trainium2, data-parallel over 8 NeuronCores.

Host side shards + permutes samples (binning is permutation invariant) into
128-sample single-label "slots" so the device never needs a per-sample label
gather, and casts logits to bf16 (ECE rel-err from bf16 ~3e-3, tolerance 2e-2).

Device per tile (128 partitions x 128 slots x 104 padded classes, bf16):
  ScalarE: one big exp(logits/T) instruction (13312 elems/partition).
  DVE:     pairwise tensor_tensor max/add trees (bf16 2x mode) for per-sample
           max-exp and softmax denominators, fast reciprocal, conf/acc, and
           15 tensor_scalar is_gt threshold compares (4x mode).
  PE:      four matmuls accumulate per-(slot, bin) sums of (conf, acc) into
           four PSUM banks across all tiles.
Each core DMAs its [4,128,480] histogram out; the host extracts the diagonal
slot blocks, reduces 8 cores' 15-bin stats, and finishes ECE in float64.
"""

import hashlib
import sys

import numpy as np

sys.path.insert(0, "/opt/trn_rl_repo")

import ml_dtypes  # noqa: E402

from concourse import bacc, bass, mybir, tile  # noqa: E402
from concourse import bass_utils  # noqa: E402

P = 128          # partitions
G = 128          # slots (groups) per tile
TILE = P * G     # samples per tile (16384)
C = 100          # classes
CP = 104         # padded class stride (4B-aligned bf16 rows, tree-friendly)
NBINS = 15
N_CORES = 8
BIG = 80.0       # pad logit; exp(-80) ~ 1.8e-35 is harmless in f32/bf16
N_TOTAL = 2_000_000

F32 = mybir.dt.float32
BF16 = mybir.dt.bfloat16
BFNP = ml_dtypes.bfloat16
AX = mybir.AxisListType
ALU = mybir.AluOpType
ACTF = mybir.ActivationFunctionType


# ---------------------------------------------------------------- host layout

def build_plan(labels: np.ndarray, n_cores: int = N_CORES):
    """Deal samples round-robin per label so every core has the same number
    of 128-sample slots per label.  Returns (slot_labels, per-core sample
    index arrays with -1 for pad rows, tile count)."""
    labels = np.asarray(labels).astype(np.int64).ravel()
    order = np.argsort(labels, kind="stable")
    sorted_labels = labels[order]
    starts = np.searchsorted(sorted_labels, np.arange(C))
    ends = np.searchsorted(sorted_labels, np.arange(C), side="right")

    slot_labels = []
    core_chunks = [[] for _ in range(n_cores)]
    for k in range(C):
        idx_k = order[starts[k]:ends[k]]
        per_core = [idx_k[c::n_cores] for c in range(n_cores)]
        max_cnt = max(len(x) for x in per_core)
        slots_k = -(-max_cnt // P) if max_cnt > 0 else 0
        if slots_k == 0:
            continue
        padded = slots_k * P
        for c in range(n_cores):
            buf = np.full(padded, -1, dtype=np.int64)
            buf[: len(per_core[c])] = per_core[c]
            core_chunks[c].append(buf)
        slot_labels.extend([k] * slots_k)

    n_slots = len(slot_labels)
    pad_slots = (-n_slots) % G
    if pad_slots:
        for c in range(n_cores):
            core_chunks[c].append(np.full(pad_slots * P, -1, dtype=np.int64))
        slot_labels.extend([0] * pad_slots)
        n_slots += pad_slots

    slot_labels = np.asarray(slot_labels, dtype=np.int64)
    core_idx = [np.concatenate(ch) for ch in core_chunks]
    T = n_slots // G
    return slot_labels, core_idx, T


def label_runs(slot_labels: np.ndarray, T: int):
    """Per tile: list of (g0, g1, k) runs of equal-label slots."""
    runs = []
    for t in range(T):
        ks = slot_labels[t * G:(t + 1) * G]
        tile_runs = []
        g0 = 0
        for g in range(1, G + 1):
            if g == G or ks[g] != ks[g0]:
                tile_runs.append((g0, g, int(ks[g0])))
                g0 = g
        runs.append(tile_runs)
    return runs


def build_core_slab(logits: np.ndarray, idx: np.ndarray,
                    slot_labels: np.ndarray) -> np.ndarray:
    """One core's [T*TILE, CP] bf16 slab in device DMA order: within a tile
    load, partition p holds G consecutive samples' class rows."""
    S = len(slot_labels)
    arr = np.full((S * P, CP), -BIG, dtype=BFNP)
    arr[:, :C] = logits[np.maximum(idx, 0)].astype(BFNP)
    pad_pos = np.nonzero(idx < 0)[0]
    if len(pad_pos):
        ks = slot_labels[pad_pos // P]
        arr[pad_pos, :C] = BFNP(-BIG)
        arr[pad_pos, ks] = BFNP(BIG)
    # [S, P, CP] slot-major -> [T, P, G, CP] DMA order
    arr = arr.reshape(S // G, G, P, CP).transpose(0, 2, 1, 3)
    return np.ascontiguousarray(arr).reshape(-1, CP)


# ------------------------------------------------------------- device program

def build_program(T: int, runs, n_cores: int = N_CORES):
    nc = bacc.Bacc("TRN2", target_bir_lowering=False, debug=False,
                   num_devices=n_cores)

    logits_d = nc.dram_tensor("logits", [T * TILE, CP], BF16,
                              kind="ExternalInput")
    tempr_d = nc.dram_tensor("tempr", [P, 1], F32, kind="ExternalInput")
    out_d = nc.dram_tensor("out", [4, P, 32 * NBINS], F32,
                           kind="ExternalOutput")

    thr_imm = [float(np.float32(BFNP(b / NBINS))) for b in range(NBINS)]

    with tile.TileContext(nc) as tc:
        with (
            tc.tile_pool(name="const", bufs=1) as const,
            tc.tile_pool(name="rawp", bufs=2) as rawp,
            tc.tile_pool(name="sb", bufs=2) as sbp,
            tc.tile_pool(name="ps", bufs=1, space="PSUM") as psp,
        ):
            tempr_t = const.tile([P, 1], F32)
            nc.sync.dma_start(tempr_t, tempr_d.ap())
            invT = const.tile([P, 1], F32)
            nc.vector.reciprocal(invT, tempr_t)

            hists = [psp.tile([P, 32 * NBINS], F32, name=f"hist{q}")
                     for q in range(4)]

            logits_ap = logits_d.ap()
            for t in range(T):
                raw = rawp.tile([P, G * CP], BF16, tag="raw", name="raw")
                src = logits_ap[t * TILE:(t + 1) * TILE,
                                :].rearrange("(p s) c -> p (s c)", p=P)
                nc.sync.dma_start(raw, src)

                # ---- ScalarE: one big exp(l/T)
                E = sbp.tile([P, G * CP], BF16, tag="E", name="E")
                nc.scalar.activation(E, raw, ACTF.Exp, scale=invT)
                E3 = E.rearrange("p (g c) -> p g c", g=G)

                # ---- DVE: max tree (bf16 2x TT ops; L3 overlaps cols 12:14)
                t1 = sbp.tile([P, G * 52], BF16, tag="t1", name="t1")
                t13 = t1.rearrange("p (g c) -> p g c", g=G)
                nc.vector.tensor_tensor(t13, E3[:, :, 0:52], E3[:, :, 52:104],
                                        op=ALU.max)
                t2 = sbp.tile([P, G * 26], BF16, tag="t2", name="t2")
                t23 = t2.rearrange("p (g c) -> p g c", g=G)
                nc.vector.tensor_tensor(t23, t13[:, :, 0:26], t13[:, :, 26:52],
                                        op=ALU.max)
                t3 = sbp.tile([P, G * 14], BF16, tag="t3", name="t3")
                t33 = t3.rearrange("p (g c) -> p g c", g=G)
                nc.vector.tensor_tensor(t33, t23[:, :, 0:14], t23[:, :, 12:26],
                                        op=ALU.max)
                emax = sbp.tile([P, G], BF16, tag="emax", name="emax")
                nc.vector.reduce_max(emax, t33, axis=AX.X)

                # ---- DVE: sum tree for denominators (no overlap allowed)
                u1 = sbp.tile([P, G * 52], BF16, tag="t1", name="u1")
                u13 = u1.rearrange("p (g c) -> p g c", g=G)
                nc.vector.tensor_tensor(u13, E3[:, :, 0:52], E3[:, :, 52:104],
                                        op=ALU.add)
                u2 = sbp.tile([P, G * 26], BF16, tag="t2", name="u2")
                u23 = u2.rearrange("p (g c) -> p g c", g=G)
                nc.vector.tensor_tensor(u23, u13[:, :, 0:26], u13[:, :, 26:52],
                                        op=ALU.add)
                u3 = sbp.tile([P, G * 14], BF16, tag="t3", name="u3")
                u33 = u3.rearrange("p (g c) -> p g c", g=G)
                nc.vector.tensor_tensor(u33[:, :, 0:12], u23[:, :, 0:12],
                                        u23[:, :, 14:26], op=ALU.add)
                nc.vector.tensor_copy(u33[:, :, 12:14], u23[:, :, 12:14])
                S = sbp.tile([P, G], F32, tag="S", name="S")
                nc.vector.reduce_sum(S, u33, axis=AX.X)

                # ---- per-sample: conf = emax / S; acc = E[label] >= emax
                R = sbp.tile([P, G], F32, tag="R", name="R")
                nc.vector.reciprocal_approx_fast(R, S)
                Rb = sbp.tile([P, G], BF16, tag="Rb", name="Rb")
                nc.vector.tensor_copy(Rb, R)
                # pack layout [conf0|acc0|conf1|acc1] (64 slots each) so each
                # matmul's lhsT is one contiguous 128-col slice.
                pack = sbp.tile([P, 2 * G], BF16, tag="pack", name="pack")
                pack4 = pack.rearrange("p (r g) -> p r g", r=4)
                confv = pack4[:, 0:3:2, :]          # [P, 2, 64] conf blocks
                nc.vector.tensor_tensor(
                    confv, emax.rearrange("p (u g) -> p u g", u=2),
                    Rb.rearrange("p (u g) -> p u g", u=2), op=ALU.mult)

                def acc_col(g):
                    return 128 * (g // 64) + 64 + (g % 64)

                for (g0, g1, k) in runs[t]:
                    for (a0, a1) in ((g0, min(g1, 64)), (max(g0, 64), g1)):
                        if a0 >= a1:
                            continue
                        lab = E3[:, a0:a1, k:k + 1].opt()
                        nc.vector.tensor_tensor(
                            pack[:, acc_col(a0):acc_col(a1 - 1) + 1], lab,
                            emax[:, a0:a1], op=ALU.is_ge)

                # ---- cumulative bin masks: 15 tensor_scalar is_gt (4x mode)
                mask = sbp.tile([P, NBINS * G], BF16, tag="mask", name="mask")
                for b in range(NBINS):
                    nc.vector.tensor_scalar(
                        mask[:, b * G:(b + 1) * G].rearrange(
                            "p (u g) -> p u g", u=2),
                        confv, thr_imm[b], None, op0=ALU.is_gt)

                # ---- histogram matmuls: 4 slot-quarters into 4 PSUM banks
                mask3 = mask.rearrange("p (b g) -> p b g", b=NBINS)
                for q in range(4):
                    u = q // 2
                    lhsT = pack[:, 128 * u:128 * u + 128]
                    rhs = mask3[:, :, 32 * q:32 * q + 32]
                    nc.tensor.matmul(hists[q], lhsT=lhsT, rhs=rhs,
                                     start=(t == 0), stop=(t == T - 1))

            # ---- finalize: dump histograms; host does the tiny reduction
            for q in range(4):
                hsb = sbp.tile([P, 32 * NBINS], F32, tag="hsb", name="hsb")
                nc.vector.tensor_copy(hsb, hists[q])
                nc.sync.dma_start(out_d.ap()[q], hsb)

    nc.compile()
    return nc


# ------------------------------------------------------------------- runner

_CACHE = {}


def _prepare(logits, labels, temperature, n_cores=N_CORES):
    labels = np.asarray(labels)
    key = hashlib.sha1(labels.tobytes()).hexdigest()
    if key in _CACHE:
        nc, slot_labels, core_idx, T = _CACHE[key]
    else:
        slot_labels, core_idx, T = build_plan(labels, n_cores)
        nc = build_program(T, label_runs(slot_labels, T), n_cores)
        _CACHE[key] = (nc, slot_labels, core_idx, T)

    logits = np.asarray(logits, dtype=np.float32)
    tempr = np.broadcast_to(
        np.asarray(temperature, np.float32).ravel()[0:1], (P, 1)).copy()
    in_maps = []
    for c in range(n_cores):
        in_maps.append({
            "tempr": tempr,
            "logits": build_core_slab(logits, core_idx[c], slot_labels),
        })
    return nc, in_maps


def finalize_host(hists, n_total=N_TOTAL):
    """hists: list of per-core [4, P, 32*NBINS] f32. Returns ECE f32 [1]."""
    j = np.arange(32)
    sc_cum = np.zeros(NBINS, np.float64)
    sa_cum = np.zeros(NBINS, np.float64)
    for h in hists:
        h5 = np.asarray(h, np.float64).reshape(4, P, NBINS, 32)
        for q in range(4):
            r0 = 32 * (q % 2)
            sc_cum += h5[q, r0 + j, :, j].sum(axis=0)
            sa_cum += h5[q, 64 + r0 + j, :, j].sum(axis=0)
    sc = sc_cum - np.concatenate([sc_cum[1:], [0.0]])
    sa = sa_cum - np.concatenate([sa_cum[1:], [0.0]])
    ece = np.abs(sc - sa).sum() / float(n_total)
    return np.asarray([ece], dtype=np.float32)


def _ensure_ntff_hook():
    """This container's antenv lacks axon_hooks; synthesize it and register
    the ctypes NTFF hook so trace=True works under axon."""
    try:
        import antenv.axon_hooks  # noqa: F401
        return
    except ImportError:
        pass
    import types

    import antenv

    mod = types.ModuleType("antenv.axon_hooks")
    _hook = [None]
    mod.set_axon_ntff_profile_hook = lambda h: _hook.__setitem__(0, h)
    mod.get_axon_ntff_profile_hook = lambda: _hook[0]
    sys.modules["antenv.axon_hooks"] = mod
    antenv.axon_hooks = mod
    try:
        from trn_agent_boot.trn_boot import _ntff_profile_via_ctypes
        mod.set_axon_ntff_profile_hook(
            _ntff_profile_via_ctypes("/opt/axon/libaxon_pjrt.so"))
    except Exception:
        pass


def run(logits, labels, temperature, n_total=None, trace=False,
        n_cores=N_CORES):
    if trace:
        _ensure_ntff_hook()
    if n_total is None:
        n_total = int(np.asarray(labels).shape[0])
    nc, in_maps = _prepare(logits, labels, temperature, n_cores)
    res = bass_utils.run_bass_kernel_spmd(
        nc, in_maps, core_ids=list(range(n_cores)), trace=trace)
    out = finalize_host([r["out"] for r in res.results], n_total)
    return out, res


def kernel(logits, labels, temperature):
    out, _ = run(logits, labels, temperature)
    return out


# revision 9
# speedup vs baseline: 1.8647x; 1.0802x over previous
"""ECE loss kernel for Trainium2, data-parallel over 8 NeuronCores.

Host side shards + permutes samples (binning is permutation invariant) into
128-sample single-label "slots" so the device never needs a per-sample label
gather, and casts logits to bf16 (ECE rel-err from bf16 ~3e-3, tolerance 2e-2).

Device per tile (128 partitions x 128 slots x 104 padded classes, bf16):
  ScalarE: one big exp(logits/T) instruction (13312 elems/partition).
  DVE:     pairwise tensor_tensor max/add trees (bf16 2x mode) for per-sample
           max-exp and softmax denominators, fast reciprocal, conf/acc, and
           15 tensor_scalar is_gt threshold compares (4x mode).
  PE:      four matmuls accumulate per-(slot, bin) sums of (conf, acc) into
           four PSUM banks across all tiles.
Each core DMAs its [4,128,480] histogram out; the host extracts the diagonal
slot blocks, reduces 8 cores' 15-bin stats, and finishes ECE in float64.
"""

import hashlib
import sys

import numpy as np

sys.path.insert(0, "/opt/trn_rl_repo")

import ml_dtypes  # noqa: E402

from concourse import bacc, bass, mybir, tile  # noqa: E402
from concourse import bass_utils  # noqa: E402

P = 128          # partitions
G = 128          # slots (groups) per tile
TILE = P * G     # samples per tile (16384)
C = 100          # classes
CP = 104         # padded class stride (4B-aligned bf16 rows, tree-friendly)
NBINS = 15
N_CORES = 8
BIG = 80.0       # pad logit; exp(-80) ~ 1.8e-35 is harmless in f32/bf16
N_TOTAL = 2_000_000

F32 = mybir.dt.float32
BF16 = mybir.dt.bfloat16
BFNP = ml_dtypes.bfloat16
AX = mybir.AxisListType
ALU = mybir.AluOpType
ACTF = mybir.ActivationFunctionType


# ---------------------------------------------------------------- host layout

def build_plan(labels: np.ndarray, n_cores: int = N_CORES):
    """Deal samples round-robin per label so every core has the same number
    of 128-sample slots per label.  Returns (slot_labels, per-core sample
    index arrays with -1 for pad rows, tile count)."""
    labels = np.asarray(labels).astype(np.int64).ravel()
    order = np.argsort(labels, kind="stable")
    sorted_labels = labels[order]
    starts = np.searchsorted(sorted_labels, np.arange(C))
    ends = np.searchsorted(sorted_labels, np.arange(C), side="right")

    slot_labels = []
    core_chunks = [[] for _ in range(n_cores)]
    for k in range(C):
        idx_k = order[starts[k]:ends[k]]
        per_core = [idx_k[c::n_cores] for c in range(n_cores)]
        max_cnt = max(len(x) for x in per_core)
        slots_k = -(-max_cnt // P) if max_cnt > 0 else 0
        if slots_k == 0:
            continue
        padded = slots_k * P
        for c in range(n_cores):
            buf = np.full(padded, -1, dtype=np.int64)
            buf[: len(per_core[c])] = per_core[c]
            core_chunks[c].append(buf)
        slot_labels.extend([k] * slots_k)

    n_slots = len(slot_labels)
    pad_slots = (-n_slots) % G
    if pad_slots:
        for c in range(n_cores):
            core_chunks[c].append(np.full(pad_slots * P, -1, dtype=np.int64))
        slot_labels.extend([0] * pad_slots)
        n_slots += pad_slots

    slot_labels = np.asarray(slot_labels, dtype=np.int64)
    core_idx = [np.concatenate(ch) for ch in core_chunks]
    T = n_slots // G
    return slot_labels, core_idx, T


def label_runs(slot_labels: np.ndarray, T: int):
    """Per tile: list of (g0, g1, k) runs of equal-label slots."""
    runs = []
    for t in range(T):
        ks = slot_labels[t * G:(t + 1) * G]
        tile_runs = []
        g0 = 0
        for g in range(1, G + 1):
            if g == G or ks[g] != ks[g0]:
                tile_runs.append((g0, g, int(ks[g0])))
                g0 = g
        runs.append(tile_runs)
    return runs


def build_core_slab(logits: np.ndarray, idx: np.ndarray,
                    slot_labels: np.ndarray) -> np.ndarray:
    """One core's [T*TILE, CP] bf16 slab in device DMA order: within a tile
    load, partition p holds G consecutive samples' class rows."""
    S = len(slot_labels)
    arr = np.full((S * P, CP), -BIG, dtype=BFNP)
    arr[:, :C] = logits[np.maximum(idx, 0)].astype(BFNP)
    pad_pos = np.nonzero(idx < 0)[0]
    if len(pad_pos):
        ks = slot_labels[pad_pos // P]
        arr[pad_pos, :C] = BFNP(-BIG)
        arr[pad_pos, ks] = BFNP(BIG)
    # [S, P, CP] slot-major -> [T, P, G, CP] DMA order
    arr = arr.reshape(S // G, G, P, CP).transpose(0, 2, 1, 3)
    return np.ascontiguousarray(arr).reshape(-1, CP)


# ------------------------------------------------------------- device program

def build_program(T: int, runs, n_cores: int = N_CORES):
    nc = bacc.Bacc("TRN2", target_bir_lowering=False, debug=False,
                   num_devices=n_cores)

    logits_d = nc.dram_tensor("logits", [T * TILE, CP], BF16,
                              kind="ExternalInput")
    tempr_d = nc.dram_tensor("tempr", [P, 1], F32, kind="ExternalInput")
    out_d = nc.dram_tensor("out", [4, P, 32 * NBINS], F32,
                           kind="ExternalOutput")

    thr_imm = [float(np.float32(BFNP(b / NBINS))) for b in range(NBINS)]

    with tile.TileContext(nc) as tc:
        with (
            tc.tile_pool(name="const", bufs=1) as const,
            tc.tile_pool(name="rawp", bufs=2) as rawp,
            tc.tile_pool(name="sb", bufs=2) as sbp,
            tc.tile_pool(name="ps", bufs=1, space="PSUM") as psp,
        ):
            tempr_t = const.tile([P, 1], F32)
            nc.sync.dma_start(tempr_t, tempr_d.ap())
            invT = const.tile([P, 1], F32)
            nc.vector.reciprocal(invT, tempr_t)

            hists = [psp.tile([P, 32 * NBINS], F32, name=f"hist{q}")
                     for q in range(4)]

            logits_ap = logits_d.ap()
            for t in range(T):
                raw = rawp.tile([P, G * CP], BF16, tag="raw", name="raw")
                E = sbp.tile([P, G * CP], BF16, tag="E", name="E")
                E3 = E.rearrange("p (g c) -> p g c", g=G)
                t1 = sbp.tile([P, G * 52], BF16, tag="t1", name="t1", bufs=1)
                t13 = t1.rearrange("p (g c) -> p g c", g=G)
                u1 = sbp.tile([P, G * 52], BF16, tag="u1", name="u1", bufs=1)
                u13 = u1.rearrange("p (g c) -> p g c", g=G)

                # Tile 0 is sub-chunked so DVE starts ~20us earlier (pipeline
                # fill); later tiles overlap DMA/exp with the previous tile.
                src = logits_ap[t * TILE:(t + 1) * TILE,
                                :].rearrange("(p s) c -> p (s c)", p=P)
                nsub = 4 if t == 0 else 1
                gs = G // nsub
                for c in range(nsub):
                    fsl = slice(c * gs * CP, (c + 1) * gs * CP)
                    gsl = slice(c * gs, (c + 1) * gs)
                    nc.sync.dma_start(raw[:, fsl], src[:, fsl])
                    nc.scalar.activation(E[:, fsl], raw[:, fsl], ACTF.Exp,
                                         scale=invT)
                    # tree L1 (bf16 2x TT): max and sum of class pairs
                    nc.vector.tensor_tensor(t13[:, gsl, :], E3[:, gsl, 0:52],
                                            E3[:, gsl, 52:104], op=ALU.max)
                    nc.vector.tensor_tensor(u13[:, gsl, :], E3[:, gsl, 0:52],
                                            E3[:, gsl, 52:104], op=ALU.add)

                # ---- max tree L2..L7 (overlapped splits keep 4B alignment)
                t2 = sbp.tile([P, G * 26], BF16, tag="t2", name="t2", bufs=1)
                t23 = t2.rearrange("p (g c) -> p g c", g=G)
                nc.vector.tensor_tensor(t23, t13[:, :, 0:26], t13[:, :, 26:52],
                                        op=ALU.max)
                t3 = sbp.tile([P, G * 14], BF16, tag="t3", name="t3", bufs=1)
                t33 = t3.rearrange("p (g c) -> p g c", g=G)
                nc.vector.tensor_tensor(t33, t23[:, :, 0:14], t23[:, :, 12:26],
                                        op=ALU.max)
                t4 = sbp.tile([P, G * 8], BF16, tag="t4", name="t4", bufs=1)
                t43 = t4.rearrange("p (g c) -> p g c", g=G)
                nc.vector.tensor_tensor(t43, t33[:, :, 0:8], t33[:, :, 6:14],
                                        op=ALU.max)
                t5 = sbp.tile([P, G * 4], BF16, tag="t5", name="t5", bufs=1)
                t53 = t5.rearrange("p (g c) -> p g c", g=G)
                nc.vector.tensor_tensor(t53, t43[:, :, 0:4], t43[:, :, 4:8],
                                        op=ALU.max)
                t6 = sbp.tile([P, G * 2], BF16, tag="t6", name="t6", bufs=1)
                t63 = t6.rearrange("p (g c) -> p g c", g=G)
                nc.vector.tensor_tensor(t63, t53[:, :, 0:2], t53[:, :, 2:4],
                                        op=ALU.max)
                emax = sbp.tile([P, G], BF16, tag="emax", name="emax", bufs=1)
                nc.vector.tensor_tensor(emax, t63[:, :, 0:1].opt(),
                                        t63[:, :, 1:2].opt(), op=ALU.max)

                # ---- sum tree L2..L7 (no overlap; odd tails pass through)
                u2 = sbp.tile([P, G * 26], BF16, tag="u2", name="u2", bufs=1)
                u23 = u2.rearrange("p (g c) -> p g c", g=G)
                nc.vector.tensor_tensor(u23, u13[:, :, 0:26], u13[:, :, 26:52],
                                        op=ALU.add)
                u3 = sbp.tile([P, G * 14], BF16, tag="u3", name="u3", bufs=1)
                u33 = u3.rearrange("p (g c) -> p g c", g=G)
                nc.vector.tensor_tensor(u33[:, :, 0:12], u23[:, :, 0:12],
                                        u23[:, :, 14:26], op=ALU.add)
                nc.vector.tensor_copy(u33[:, :, 12:14], u23[:, :, 12:14])
                u4 = sbp.tile([P, G * 8], BF16, tag="u4", name="u4", bufs=1)
                u43 = u4.rearrange("p (g c) -> p g c", g=G)
                nc.vector.tensor_tensor(u43[:, :, 0:6], u33[:, :, 0:6],
                                        u33[:, :, 8:14], op=ALU.add)
                nc.vector.tensor_copy(u43[:, :, 6:8], u33[:, :, 6:8])
                u5 = sbp.tile([P, G * 4], BF16, tag="u5", name="u5", bufs=1)
                u53 = u5.rearrange("p (g c) -> p g c", g=G)
                nc.vector.tensor_tensor(u53, u43[:, :, 0:4], u43[:, :, 4:8],
                                        op=ALU.add)
                u6 = sbp.tile([P, G * 2], BF16, tag="u6", name="u6", bufs=1)
                u63 = u6.rearrange("p (g c) -> p g c", g=G)
                nc.vector.tensor_tensor(u63, u53[:, :, 0:2], u53[:, :, 2:4],
                                        op=ALU.add)
                S = sbp.tile([P, G], F32, tag="S", name="S", bufs=1)
                nc.vector.tensor_tensor(S, u63[:, :, 0:1].opt(),
                                        u63[:, :, 1:2].opt(), op=ALU.add)

                # ---- per-sample: conf = emax / S; acc = E[label] >= emax
                R = sbp.tile([P, G], F32, tag="R", name="R", bufs=1)
                nc.vector.reciprocal_approx_fast(R, S)
                Rb = sbp.tile([P, G], BF16, tag="Rb", name="Rb", bufs=1)
                nc.vector.tensor_copy(Rb, R)
                conf2 = sbp.tile([P, G], BF16, tag="conf2", name="conf2", bufs=1)
                nc.vector.tensor_tensor(conf2, emax, Rb, op=ALU.mult)
                # pack layout [conf0|acc0|conf1|acc1] (64 slots each) so each
                # matmul's lhsT is one contiguous 128-col slice.
                pack = sbp.tile([P, 2 * G], BF16, tag="pack", name="pack")
                pack4 = pack.rearrange("p (r g) -> p r g", r=4)
                nc.vector.tensor_copy(
                    pack4[:, 0:3:2, :], conf2.rearrange("p (u g) -> p u g",
                                                        u=2))

                def acc_col(g):
                    return 128 * (g // 64) + 64 + (g % 64)

                for (g0, g1, k) in runs[t]:
                    for (a0, a1) in ((g0, min(g1, 64)), (max(g0, 64), g1)):
                        if a0 >= a1:
                            continue
                        lab = E3[:, a0:a1, k:k + 1].opt()
                        nc.vector.tensor_tensor(
                            pack[:, acc_col(a0):acc_col(a1 - 1) + 1], lab,
                            emax[:, a0:a1], op=ALU.is_ge)

                # ---- cumulative bin masks: 15 tensor_scalar is_gt (4x mode)
                mask = sbp.tile([P, NBINS * G], BF16, tag="mask", name="mask")
                for b in range(NBINS):
                    nc.vector.tensor_scalar(mask[:, b * G:(b + 1) * G],
                                            conf2, thr_imm[b], None,
                                            op0=ALU.is_gt)

                # ---- histogram matmuls: 4 slot-quarters into 4 PSUM banks
                mask3 = mask.rearrange("p (b g) -> p b g", b=NBINS)
                for q in range(4):
                    u = q // 2
                    lhsT = pack[:, 128 * u:128 * u + 128]
                    rhs = mask3[:, :, 32 * q:32 * q + 32]
                    nc.tensor.matmul(hists[q], lhsT=lhsT, rhs=rhs,
                                     start=(t == 0), stop=(t == T - 1))

            # ---- finalize: dump histograms; host does the tiny reduction
            for q in range(4):
                hsb = sbp.tile([P, 32 * NBINS], F32, tag="hsb", name="hsb")
                nc.scalar.copy(hsb, hists[q])
                nc.sync.dma_start(out_d.ap()[q], hsb)

    nc.compile()
    return nc


# ------------------------------------------------------------------- runner

_CACHE = {}


def _prepare(logits, labels, temperature, n_cores=N_CORES):
    labels = np.asarray(labels)
    key = hashlib.sha1(labels.tobytes()).hexdigest()
    if key in _CACHE:
        nc, slot_labels, core_idx, T = _CACHE[key]
    else:
        slot_labels, core_idx, T = build_plan(labels, n_cores)
        nc = build_program(T, label_runs(slot_labels, T), n_cores)
        _CACHE[key] = (nc, slot_labels, core_idx, T)

    logits = np.asarray(logits, dtype=np.float32)
    tempr = np.broadcast_to(
        np.asarray(temperature, np.float32).ravel()[0:1], (P, 1)).copy()
    in_maps = []
    for c in range(n_cores):
        in_maps.append({
            "tempr": tempr,
            "logits": build_core_slab(logits, core_idx[c], slot_labels),
        })
    return nc, in_maps


def finalize_host(hists, n_total=N_TOTAL):
    """hists: list of per-core [4, P, 32*NBINS] f32. Returns ECE f32 [1]."""
    j = np.arange(32)
    sc_cum = np.zeros(NBINS, np.float64)
    sa_cum = np.zeros(NBINS, np.float64)
    for h in hists:
        h5 = np.asarray(h, np.float64).reshape(4, P, NBINS, 32)
        for q in range(4):
            r0 = 32 * (q % 2)
            sc_cum += h5[q, r0 + j, :, j].sum(axis=0)
            sa_cum += h5[q, 64 + r0 + j, :, j].sum(axis=0)
    sc = sc_cum - np.concatenate([sc_cum[1:], [0.0]])
    sa = sa_cum - np.concatenate([sa_cum[1:], [0.0]])
    ece = np.abs(sc - sa).sum() / float(n_total)
    return np.asarray([ece], dtype=np.float32)


def _ensure_ntff_hook():
    """This container's antenv lacks axon_hooks; synthesize it and register
    the ctypes NTFF hook so trace=True works under axon."""
    try:
        import antenv.axon_hooks  # noqa: F401
        return
    except ImportError:
        pass
    import types

    import antenv

    mod = types.ModuleType("antenv.axon_hooks")
    _hook = [None]
    mod.set_axon_ntff_profile_hook = lambda h: _hook.__setitem__(0, h)
    mod.get_axon_ntff_profile_hook = lambda: _hook[0]
    sys.modules["antenv.axon_hooks"] = mod
    antenv.axon_hooks = mod
    try:
        from trn_agent_boot.trn_boot import _ntff_profile_via_ctypes
        mod.set_axon_ntff_profile_hook(
            _ntff_profile_via_ctypes("/opt/axon/libaxon_pjrt.so"))
    except Exception:
        pass


def run(logits, labels, temperature, n_total=None, trace=False,
        n_cores=N_CORES):
    if trace:
        _ensure_ntff_hook()
    if n_total is None:
        n_total = int(np.asarray(labels).shape[0])
    nc, in_maps = _prepare(logits, labels, temperature, n_cores)
    res = bass_utils.run_bass_kernel_spmd(
        nc, in_maps, core_ids=list(range(n_cores)), trace=trace)
    out = finalize_host([r["out"] for r in res.results], n_total)
    return out, res


def kernel(logits, labels, temperature):
    out, _ = run(logits, labels, temperature)
    return out
